# revision 7
# baseline (speedup 1.0000x reference)
"""MiniSTU Trainium2 kernel (8 NeuronCores, Bass/Tile).

Math: the reference's FFT convolution + einsum collapses to
    y[b,l,o] = sum_g sum_{t<=l} phi_eff_g[l-t] * (x[b,t] @ M_g)[o]
over g in the 48 (filter k, sign) pairs, where phi_eff carries the
(-1)^s alternation for the minus branch (the two sgn factors in the
reference combine to (-1)^(l-t), i.e. an alternating filter).

Device algorithm per core (6 pairs per core, filter-dim sharding):
  stage 1: Z_g[t, (b,o)] = xT_tile.T @ M_g       (PE, f32r)
  stage 2: y[c] += Toeplitz(phi_eff_g)[c-cp].T @ Z_g[cp]   (PE, f32r)
Toeplitz blocks are expanded on host from phi. The 8 per-core partial
outputs are summed on host (the gather for this sharding).
"""

import numpy as np

import concourse.bass as bass
import concourse.tile as tile
from concourse import mybir
from concourse.bass_utils import run_bass_kernel_spmd
from concourse.vector_clock import ScopedClock

L = 2048
K = 24
I = 256
O = 256
B = 2
TS = 128          # tile size along sequence
CT = L // TS      # 16 sequence tiles
NP = 6            # (k, sign) pairs per core
N_CORES = 8
BO = B * O        # 512 fused (b, o) columns
F32 = mybir.dt.float32
F32R = mybir.dt.float32r


# ---------------------------------------------------------------------------
# Workarounds for this container's walrus: it rejects any instruction that
# carries more than one sync-wait command.
# ---------------------------------------------------------------------------

def _split_sync_waits(nc, max_waits=1):
    """Hoist extra sem-waits onto same-engine NOPs inserted right before the
    offending instruction; queue order keeps the semantics identical."""
    for f in nc.m.functions:
        for blk in f.blocks:
            insts = list(blk.instructions)
            out = []
            changed = False
            for inst in insts:
                si = getattr(inst, "sync_info", None)
                waits = list(si.on_wait) if si is not None else []
                if len(waits) > max_waits:
                    changed = True
                    extra, keep = waits[:-max_waits], waits[-max_waits:]
                    for j in range(0, len(extra), max_waits):
                        nop = mybir.InstNoOp(
                            name=nc.get_next_instruction_name(), ins=[], outs=[]
                        )
                        nop.engine = inst.engine
                        nop.sync_info = mybir.SyncInfo(
                            on_wait=extra[j : j + max_waits], on_update=[]
                        )
                        out.append(nop)
                    inst.sync_info = mybir.SyncInfo(
                        on_wait=keep, on_update=list(si.on_update)
                    )
                out.append(inst)
            if changed:
                blk.instructions = out


class _TC(tile.TileContext):
    """TileContext whose tail drain spreads its waits over 1-wait NOPs."""

    def _drain_and_barrier(self, tick_clock, wait_clock):
        nc = self.nc
        nop_inst = nc.sync.nop()
        wait_clock.add_sem_waits(
            nop_inst.ins, ScopedClock({None: tick_clock.global_clock})
        )
        si = nop_inst.ins.sync_info
        if si is not None and len(si.on_wait) > 1:
            waits = list(si.on_wait)
            nop_inst.ins.sync_info = mybir.SyncInfo(
                on_wait=waits[:1], on_update=list(si.on_update)
            )
            for w in waits[1:]:
                extra = nc.sync.nop().ins
                extra.sync_info = mybir.SyncInfo(on_wait=[w], on_update=[])
        nc.sync.drain()
        nc.all_engine_barrier()
        assert self.sems is not None
        popped = nc._tile_sem_poison_stack.pop()
        assert popped is self._sem_poison
        nc.clear_and_free_semaphores(list(self.sems.allocated().values()))
        nc.all_engine_barrier()


# ---------------------------------------------------------------------------
# Device program (identical on all 8 cores; per-core data differs)
# ---------------------------------------------------------------------------

def _build_nc():
    nc = bass.Bass("TRN2", target_bir_lowering=False, debug=False,
                   num_devices=N_CORES)
    # x chunked per sequence tile: [b, ic, cp, i, t_in_tile]
    xT_d = nc.dram_tensor("xT", [B, 2, CT, TS, TS], F32R, kind="ExternalInput")
    # M fused per pair-pair: [pp, ic, i, (p0 o | p1 o)]
    m_d = nc.dram_tensor("m", [NP // 2, 2, TS, 2 * O], F32R, kind="ExternalInput")
    tb_d = nc.dram_tensor("tb", [NP, CT, TS, TS], F32R, kind="ExternalInput")
    yp_d = nc.dram_tensor("yp", [CT, TS, BO], F32, kind="ExternalOutput")

    with _TC(nc) as tc:
        with (
            tc.tile_pool(name="const", bufs=1) as cpool,
            tc.tile_pool(name="ys", bufs=1) as ypool,
            tc.tile_pool(name="z", bufs=12) as zpool,
            tc.tile_pool(name="ps1", bufs=4, space="PSUM") as ps1,
            tc.tile_pool(name="ps2", bufs=4, space="PSUM") as ps2,
        ):
            ms = [[cpool.tile([TS, 2 * O], F32R, tag=f"m{pp}{ic}", name=f"m{pp}{ic}")
                   for ic in range(2)] for pp in range(NP // 2)]
            for pp in range(NP // 2):
                for ic in range(2):
                    nc.sync.dma_start(ms[pp][ic][:], m_d[pp, ic])
            xs = [[[cpool.tile([TS, TS], F32R, tag=f"x{b}{ic}{cp}", name=f"x{b}{ic}{cp}")
                    for cp in range(CT)] for ic in range(2)] for b in range(B)]
            for cp in range(CT):         # cp-major: earliest-needed first
                for b in range(B):
                    for ic in range(2):
                        eng = nc.sync if (b + ic) % 2 == 0 else nc.gpsimd
                        eng.dma_start(xs[b][ic][cp][:], xT_d[b, ic, cp])
            tbs = [[cpool.tile([TS, TS], F32R, tag=f"t{p}{d}", name=f"t{p}{d}") for d in range(CT)]
                   for p in range(NP)]
            for d in range(CT):          # d-major: earliest-needed first
                for p in range(NP):
                    eng = nc.sync if (d * NP + p) % 2 == 0 else nc.gpsimd
                    eng.dma_start(tbs[p][d][:], tb_d[p, d])

            y_sb = [ypool.tile([TS, BO], F32, tag=f"y{c}", name=f"ysb{c}") for c in range(CT)]

            for cp in range(CT):
                # stage 1: Z for a pair-pair of filters, both batches,
                # N=512 matmuls: psum(b,pp)[t, (p0 o | p1 o)]
                zts = [zpool.tile([TS, BO], F32R, tag="z", name=f"z{cp}_{p}")
                       for p in range(NP)]
                for pp in range(NP // 2):
                    pss = []
                    for b in range(B):
                        ps = ps1.tile([TS, BO], F32, tag="s1")
                        for ic in range(2):
                            nc.tensor.matmul(
                                ps[:],
                                xs[b][ic][cp][:],
                                ms[pp][ic][:],
                                start=(ic == 0),
                                stop=(ic == 1),
                            )
                        pss.append(ps)
                    # gather into per-pair Z[t, (b0 o | b1 o)]
                    for h in range(2):
                        z = zts[2 * pp + h]
                        for b in range(B):
                            nc.vector.tensor_copy(
                                z[:, b * O:(b + 1) * O],
                                pss[b][:, h * O:(h + 1) * O],
                            )
                # stage 2: scatter this tile's contribution to all c >= cp
                for c in range(cp, CT):
                    yps = ps2.tile([TS, BO], F32, tag="s2")
                    for p in range(NP):
                        nc.tensor.matmul(
                            yps[:],
                            tbs[p][c - cp][:],
                            zts[p][:],
                            start=(p == 0),
                            stop=(p == NP - 1),
                        )
                    if cp == 0:
                        nc.vector.tensor_copy(y_sb[c][:], yps[:])
                    else:
                        nc.vector.tensor_add(y_sb[c][:], y_sb[c][:], yps[:])
                    if c == cp:  # y_sb[cp] just received its last contribution
                        nc.sync.dma_start(yp_d[cp], y_sb[cp][:])

    _split_sync_waits(nc)
    return nc


# ---------------------------------------------------------------------------
# Host side: input staging, sharding, gather
# ---------------------------------------------------------------------------

def _build_toeplitz(phi_eff):
    """tb[d, t, l] = phi_eff[d*TS + l - t] (0 where the index is negative)."""
    pad = np.zeros(L + TS - 1, np.float32)
    pad[TS - 1:] = phi_eff
    d = np.arange(CT)[:, None, None]
    t = np.arange(TS)[None, :, None]
    l = np.arange(TS)[None, None, :]
    return pad[d * TS + l - t + TS - 1]


_last_in_maps = None  # stashed for external profiling harnesses


def kernel(x, phi, M_phi_plus, M_phi_minus):
    global _last_in_maps
    x = np.asarray(x, np.float32)
    phi = np.asarray(phi, np.float32)
    Mp = np.asarray(M_phi_plus, np.float32)
    Mm = np.asarray(M_phi_minus, np.float32)

    # [b, ic, cp, i, t]: per-sequence-tile chunks of x^T
    xT = np.ascontiguousarray(
        x.reshape(B, CT, TS, 2, TS).transpose(0, 3, 1, 4, 2)
    )
    sgn = ((-1.0) ** np.arange(L)).astype(np.float32)

    m_all = np.empty((2 * K, 2, TS, O), np.float32)
    tb_all = np.empty((2 * K, CT, TS, TS), np.float32)
    for g in range(2 * K):
        k, s = g // 2, g % 2
        m_all[g] = (Mm if s else Mp)[k].reshape(2, TS, O)
        phi_eff = phi[:, k] * (sgn if s else 1.0)
        tb_all[g] = _build_toeplitz(phi_eff)
    # fuse pair-pairs into columns: [pp, ic, i, (g0 o | g1 o)] per core
    m_fused = np.concatenate(
        [m_all[0::2], m_all[1::2]], axis=3
    )  # [24, 2, TS, 2*O] where row j holds (g=2j | g=2j+1)

    nc = _build_nc()
    in_maps = []
    for core in range(N_CORES):
        gs = slice(core * NP, (core + 1) * NP)
        pps = slice(core * (NP // 2), (core + 1) * (NP // 2))
        in_maps.append({
            "xT": xT,
            "m": np.ascontiguousarray(m_fused[pps]),
            "tb": np.ascontiguousarray(tb_all[gs]),
        })
    _last_in_maps = in_maps
    res = run_bass_kernel_spmd(nc, in_maps, list(range(N_CORES)))
    y = np.zeros((CT, TS, B, O), np.float64)
    for core in range(N_CORES):
        y += res.results[core]["yp"].reshape(CT, TS, B, O)
    return np.ascontiguousarray(
        y.transpose(2, 0, 1, 3).reshape(B, L, O)
    ).astype(np.float32)


# revision 8
# speedup vs baseline: 1.1781x; 1.1781x over previous
"""MiniSTU Trainium2 kernel (8 NeuronCores, Bass/Tile).

Math: the reference's FFT convolution + einsum collapses to
    y[b,l,o] = sum_g sum_{t<=l} phi_eff_g[l-t] * (x[b,t] @ M_g)[o]
over g in the 48 (filter k, sign) pairs, where phi_eff carries the
(-1)^s alternation for the minus branch (the two sgn factors in the
reference combine to (-1)^(l-t), i.e. an alternating filter).

Device algorithm per core (6 pairs per core, filter-dim sharding):
  stage 1: Z_g[t, (b,o)] = xT_tile.T @ M_g       (PE, f32r)
  stage 2: y[c] += Toeplitz(phi_eff_g)[c-cp].T @ Z_g[cp]   (PE, f32r)
Toeplitz blocks are expanded on host from phi. The 8 per-core partial
outputs are summed on host (the gather for this sharding).
"""

import numpy as np

import concourse.bass as bass
import concourse.tile as tile
from concourse import mybir
from concourse.bass_utils import run_bass_kernel_spmd
from concourse.vector_clock import ScopedClock

L = 2048
K = 24
I = 256
O = 256
B = 2
TS = 128          # tile size along sequence
CT = L // TS      # 16 sequence tiles
NP = 6            # (k, sign) pairs per core
N_CORES = 8
BO = B * O        # 512 fused (b, o) columns
F32 = mybir.dt.float32
F32R = mybir.dt.float32r


# ---------------------------------------------------------------------------
# Workarounds for this container's walrus: it rejects any instruction that
# carries more than one sync-wait command.
# ---------------------------------------------------------------------------

def _split_sync_waits(nc, max_waits=1):
    """Hoist extra sem-waits onto same-engine NOPs inserted right before the
    offending instruction; queue order keeps the semantics identical."""
    for f in nc.m.functions:
        for blk in f.blocks:
            insts = list(blk.instructions)
            out = []
            changed = False
            for inst in insts:
                si = getattr(inst, "sync_info", None)
                waits = list(si.on_wait) if si is not None else []
                if len(waits) > max_waits:
                    changed = True
                    extra, keep = waits[:-max_waits], waits[-max_waits:]
                    for j in range(0, len(extra), max_waits):
                        nop = mybir.InstNoOp(
                            name=nc.get_next_instruction_name(), ins=[], outs=[]
                        )
                        nop.engine = inst.engine
                        nop.sync_info = mybir.SyncInfo(
                            on_wait=extra[j : j + max_waits], on_update=[]
                        )
                        out.append(nop)
                    inst.sync_info = mybir.SyncInfo(
                        on_wait=keep, on_update=list(si.on_update)
                    )
                out.append(inst)
            if changed:
                blk.instructions = out


class _TC(tile.TileContext):
    """TileContext whose tail drain spreads its waits over 1-wait NOPs."""

    def _drain_and_barrier(self, tick_clock, wait_clock):
        nc = self.nc
        nop_inst = nc.sync.nop()
        wait_clock.add_sem_waits(
            nop_inst.ins, ScopedClock({None: tick_clock.global_clock})
        )
        si = nop_inst.ins.sync_info
        if si is not None and len(si.on_wait) > 1:
            waits = list(si.on_wait)
            nop_inst.ins.sync_info = mybir.SyncInfo(
                on_wait=waits[:1], on_update=list(si.on_update)
            )
            for w in waits[1:]:
                extra = nc.sync.nop().ins
                extra.sync_info = mybir.SyncInfo(on_wait=[w], on_update=[])
        nc.sync.drain()
        nc.all_engine_barrier()
        assert self.sems is not None
        popped = nc._tile_sem_poison_stack.pop()
        assert popped is self._sem_poison
        nc.clear_and_free_semaphores(list(self.sems.allocated().values()))
        nc.all_engine_barrier()


# ---------------------------------------------------------------------------
# Device program (identical on all 8 cores; per-core data differs)
# ---------------------------------------------------------------------------

def _build_nc():
    nc = bass.Bass("TRN2", target_bir_lowering=False, debug=False,
                   num_devices=N_CORES)
    # x batched per sequence tile: [cp, i, (b, ic, t)]
    xT_d = nc.dram_tensor("xT", [CT, TS, B * 2 * TS], F32R, kind="ExternalInput")
    # M fused per pair-pair: [pp, ic, i, (p0 o | p1 o)]
    m_d = nc.dram_tensor("m", [NP // 2, 2, TS, 2 * O], F32R, kind="ExternalInput")
    # Toeplitz blocks batched per diagonal: [d, t, (p, l)]
    tb_d = nc.dram_tensor("tb", [CT, TS, NP * TS], F32R, kind="ExternalInput")
    yp_d = nc.dram_tensor("yp", [CT, TS, BO], F32, kind="ExternalOutput")

    with _TC(nc) as tc:
        with (
            tc.tile_pool(name="const", bufs=1) as cpool,
            tc.tile_pool(name="ys", bufs=1) as ypool,
            tc.tile_pool(name="z", bufs=12) as zpool,
            tc.tile_pool(name="ps1", bufs=4, space="PSUM") as ps1,
            tc.tile_pool(name="ps2", bufs=4, space="PSUM") as ps2,
        ):
            ms = [[cpool.tile([TS, 2 * O], F32R, tag=f"m{pp}{ic}", name=f"m{pp}{ic}")
                   for ic in range(2)] for pp in range(NP // 2)]
            for pp in range(NP // 2):
                for ic in range(2):
                    nc.sync.dma_start(ms[pp][ic][:], m_d[pp, ic])
            xs = [cpool.tile([TS, B * 2 * TS], F32R, tag=f"x{cp}", name=f"x{cp}")
                  for cp in range(CT)]
            tbs = [cpool.tile([TS, NP * TS], F32R, tag=f"t{d}", name=f"t{d}")
                   for d in range(CT)]
            # consumption order: x[s] and tb[s] interleaved across both queues
            for s in range(CT):
                nc.gpsimd.dma_start(xs[s][:], xT_d[s])
                nc.sync.dma_start(tbs[s][:], tb_d[s])

            y_sb = [ypool.tile([TS, BO], F32, tag=f"y{c}", name=f"ysb{c}") for c in range(CT)]

            for cp in range(CT):
                # stage 1: Z for a pair-pair of filters, both batches,
                # N=512 matmuls: psum(b,pp)[t, (p0 o | p1 o)]
                zts = [zpool.tile([TS, BO], F32R, tag="z", name=f"z{cp}_{p}")
                       for p in range(NP)]
                for pp in range(NP // 2):
                    pss = []
                    for b in range(B):
                        ps = ps1.tile([TS, BO], F32, tag="s1")
                        for ic in range(2):
                            nc.tensor.matmul(
                                ps[:],
                                xs[cp][:, (b * 2 + ic) * TS:(b * 2 + ic + 1) * TS],
                                ms[pp][ic][:],
                                start=(ic == 0),
                                stop=(ic == 1),
                            )
                        pss.append(ps)
                    # gather into per-pair Z[t, (b0 o | b1 o)]
                    for h in range(2):
                        z = zts[2 * pp + h]
                        for b in range(B):
                            nc.vector.tensor_copy(
                                z[:, b * O:(b + 1) * O],
                                pss[b][:, h * O:(h + 1) * O],
                            )
                # stage 2: scatter this tile's contribution to all c >= cp
                for c in range(cp, CT):
                    yps = ps2.tile([TS, BO], F32, tag="s2")
                    for p in range(NP):
                        nc.tensor.matmul(
                            yps[:],
                            tbs[c - cp][:, p * TS:(p + 1) * TS],
                            zts[p][:],
                            start=(p == 0),
                            stop=(p == NP - 1),
                        )
                    if cp == 0:
                        nc.vector.tensor_copy(y_sb[c][:], yps[:])
                    else:
                        nc.vector.tensor_add(y_sb[c][:], y_sb[c][:], yps[:])
                    if c == cp:  # y_sb[cp] just received its last contribution
                        nc.sync.dma_start(yp_d[cp], y_sb[cp][:])

    _split_sync_waits(nc)
    return nc


# ---------------------------------------------------------------------------
# Host side: input staging, sharding, gather
# ---------------------------------------------------------------------------

def _build_toeplitz(phi_eff):
    """tb[d, t, l] = phi_eff[d*TS + l - t] (0 where the index is negative)."""
    pad = np.zeros(L + TS - 1, np.float32)
    pad[TS - 1:] = phi_eff
    d = np.arange(CT)[:, None, None]
    t = np.arange(TS)[None, :, None]
    l = np.arange(TS)[None, None, :]
    return pad[d * TS + l - t + TS - 1]


_last_in_maps = None  # stashed for external profiling harnesses


def kernel(x, phi, M_phi_plus, M_phi_minus):
    global _last_in_maps
    x = np.asarray(x, np.float32)
    phi = np.asarray(phi, np.float32)
    Mp = np.asarray(M_phi_plus, np.float32)
    Mm = np.asarray(M_phi_minus, np.float32)

    # [cp, i, (b, ic, t)]: per-sequence-tile chunks of x^T, one DMA per cp
    xT = np.ascontiguousarray(
        x.reshape(B, CT, TS, 2, TS).transpose(1, 4, 0, 3, 2)
    ).reshape(CT, TS, B * 2 * TS)
    sgn = ((-1.0) ** np.arange(L)).astype(np.float32)

    m_all = np.empty((2 * K, 2, TS, O), np.float32)
    tb_all = np.empty((2 * K, CT, TS, TS), np.float32)
    for g in range(2 * K):
        k, s = g // 2, g % 2
        m_all[g] = (Mm if s else Mp)[k].reshape(2, TS, O)
        phi_eff = phi[:, k] * (sgn if s else 1.0)
        tb_all[g] = _build_toeplitz(phi_eff)
    # fuse pair-pairs into columns: [pp, ic, i, (g0 o | g1 o)] per core
    m_fused = np.concatenate(
        [m_all[0::2], m_all[1::2]], axis=3
    )  # [24, 2, TS, 2*O] where row j holds (g=2j | g=2j+1)

    nc = _build_nc()
    in_maps = []
    for core in range(N_CORES):
        gs = slice(core * NP, (core + 1) * NP)
        pps = slice(core * (NP // 2), (core + 1) * (NP // 2))
        # [d, t, (p, l)] diagonal-major Toeplitz blocks for this core
        tb_core = np.ascontiguousarray(
            tb_all[gs].transpose(1, 2, 0, 3)
        ).reshape(CT, TS, NP * TS)
        in_maps.append({
            "xT": xT,
            "m": np.ascontiguousarray(m_fused[pps]),
            "tb": tb_core,
        })
    _last_in_maps = in_maps
    res = run_bass_kernel_spmd(nc, in_maps, list(range(N_CORES)))
    y = np.zeros((CT, TS, B, O), np.float64)
    for core in range(N_CORES):
        y += res.results[core]["yp"].reshape(CT, TS, B, O)
    return np.ascontiguousarray(
        y.transpose(2, 0, 1, 3).reshape(B, L, O)
    ).astype(np.float32)
